# revision 45
# baseline (speedup 1.0000x reference)
"""Trainium2 Bass kernel for nn_Bfly_BertIntermediate (butterfly MLP + bias + gelu).

Algorithm ("Monarch" factorization of the 10-layer butterfly over N=1024):
  layers 0..6  (strides 1..64) == block-diagonal A: 8 blocks of 128x128 per stack
  layers 7..9  (strides 128..512) == per-residue-128 mixing B: 8x8 over block idx j

Device pipeline per core (2048 tokens, data-parallel over 8 cores):
  DMA in token-major -> PE transpose to feature-major -> stage-A matmuls ->
  DVE cast-copy f32->bf16 -> SBUF->SBUF DMA partition shuffle (block->residue
  grouping) -> stage-B flipped matmuls (token-major PSUM out, bias preloaded
  via K=1 ones x bias matmul) -> ScalarE gelu PSUM->SBUF -> SWDGE cast DMA
  (bf16->f32) to HBM.

A/B/bias are a tiny host-side repacking of the twiddle weights (~0.1 GFLOP).
"""
import numpy as np

import concourse.bass as bass
import concourse.mybir as mybir
import concourse.tile as tile
from concourse import bacc, bass_utils
from concourse.tile_rust import add_dep_helper

# problem shapes (hardcoded per harness contract)
B_, S_, N_ = 4, 4096, 1024
NSTACKS, LOG_N = 4, 10
SPLIT = 7                      # layers 0..6 -> A, 7..9 -> B
NJ = 8                         # 1024/128 blocks per stack
NG = 8                         # residue groups of 16
NCORES = 8
TOK = B_ * S_                  # 16384 tokens
TPC = TOK // NCORES            # 2048 tokens per core
ST_TOK = 512                   # supertile tokens
NSUP = TPC // ST_TOK           # 4 supertiles
NCH = ST_TOK // 128            # 4 chunks of 128 tokens

F32 = mybir.dt.float32
F32R = mybir.dt.float32r
BF16 = mybir.dt.bfloat16

# knobs
STAGE_A_DT = "f32r"            # "f32" (exact, 4 cyc/row) or "f32r" (1 cyc/row @ N>=512)
OUT_DT = "bf16"                # gelu output dtype before the cast-DMA to f32 HBM
GELU = "gelu"                  # "copy" for CoreSim (Gelu unimplemented there)


# ---------------------------------------------------------------- host factor
def _apply_layers(h, twiddle, layers):
    T, nstacks, n = h.shape
    for i in layers:
        stride = 1 << i
        nblk = n // (2 * stride)
        hr = h.reshape(T, nstacks, nblk, 2, stride)
        t = twiddle[:, i].reshape(nstacks, nblk, stride, 2, 2)
        hr = np.einsum('kbpoi,Tkbip->Tkbop', t, hr)
        h = hr.reshape(T, nstacks, n)
    return h


def _factor_weights(twiddle, bias):
    tw = np.asarray(twiddle, np.float64)
    eye = np.broadcast_to(np.eye(N_)[:, None, :], (N_, NSTACKS, N_)).copy()
    hA = _apply_layers(eye, tw, range(SPLIT))
    A_full = hA.transpose(1, 2, 0)          # [k, out_feat, in_feat]
    hB = _apply_layers(eye, tw, range(SPLIT, LOG_N))
    B_full = hB.transpose(1, 2, 0)

    # At[k, j, c, m] = A[k,j][m, c]  (lhsT layout: [K=c, M=m])
    At = np.empty((NSTACKS, NJ, 128, 128), np.float32)
    for j in range(NJ):
        blk = A_full[:, 128 * j:128 * (j + 1), 128 * j:128 * (j + 1)]
        At[:, j] = blk.transpose(0, 2, 1)

    # Bmat[k, r, j', j] = B_full[k, 128j'+r, 128j+r]
    jj = 128 * np.arange(NJ)
    Bmat = np.empty((NSTACKS, 128, NJ, NJ))
    for r in range(128):
        Bmat[:, r] = B_full[:, jj[:, None] + r, jj[None, :] + r]

    # Bw[k, g, q=(16j+a), of=(16j'+a)] = Bmat[k, 16g+a, j', j]
    Bw = np.zeros((NSTACKS, NG, 128, 128), np.float32)
    j16 = 16 * np.arange(NJ)
    for k in range(NSTACKS):
        for g in range(NG):
            for a in range(16):
                Bw[k, g][np.ix_(j16 + a, j16 + a)] = Bmat[k, 16 * g + a].T

    # bias4[k, g, c*128 + 16j'+a] = bias[1024k + 128j' + 16g + a], repeated 4 c's
    bias = np.asarray(bias, np.float64)
    b1 = np.empty((NSTACKS, NG, 128))
    for k in range(NSTACKS):
        for g in range(NG):
            for jp in range(NJ):
                b1[k, g, 16 * jp:16 * jp + 16] = \
                    bias[1024 * k + 128 * jp + 16 * g + np.arange(16)]
    return At, Bw.astype(np.float32), b1.astype(np.float32)


# ---------------------------------------------------------------- device IR
def build_kernel():
    nc = bacc.Bacc()
    a_dt0 = F32R if STAGE_A_DT == "f32r" else F32
    x_d = nc.dram_tensor("x", [TPC, N_], F32, kind="ExternalInput")
    At_d = nc.dram_tensor("At", [NSTACKS, NJ, 128, 128], a_dt0, kind="ExternalInput")
    Bw_d = nc.dram_tensor("Bw", [NSTACKS, NG, 128, 128], BF16, kind="ExternalInput")
    b4_d = nc.dram_tensor("bias4", [NSTACKS, NG, 128], BF16, kind="ExternalInput")
    id_d = nc.dram_tensor("ident", [128, 128], F32, kind="ExternalInput")
    out_d = nc.dram_tensor("out", [TPC, NSTACKS * N_], F32, kind="ExternalOutput")

    a_dt = F32R if STAGE_A_DT == "f32r" else F32
    out_dt = BF16 if OUT_DT == "bf16" else F32

    with tile.TileContext(nc) as tc:
        with (
            tc.tile_pool(name="consts", bufs=1) as consts,
            tc.tile_pool(name="xtm", bufs=2) as xtm_p,
            tc.tile_pool(name="xfm", bufs=2) as xfm_p,
            tc.tile_pool(name="ybf", bufs=2) as ybf_p,
            tc.tile_pool(name="ysh", bufs=2) as ysh_p,
            tc.tile_pool(name="outb", bufs=2) as outb_p,
            tc.tile_pool(name="ps_t", bufs=2, space="PSUM") as pst_p,
            tc.tile_pool(name="ps_y", bufs=2, space="PSUM") as psy_p,
            tc.tile_pool(name="ps_o", bufs=4, space="PSUM") as pso_p,
        ):
            def wait_funnel(engine, *deps):
                """NOP on `engine` that sync-depends on `deps` (BassInstructions).
                Absorbs multi-semaphore waits so the following DMA (1-wait
                budget in its ISA struct) needs none."""
                nop = nc.engines[engine].nop()
                for d in deps:
                    if d is not None:
                        add_dep_helper(nop.ins, d.ins,
                                       reason="dma wait funnel")
                return nop

            def pe_funnel(ap_a, ap_b):
                """Dummy bf16 matmul on PE reading ap_a/ap_b so that the
                multi-semaphore waits land on a bf16 MM (separate LDWEIGHTS,
                larger wait budget) instead of a self-loading f32/f32r/transpose
                matmul (1-wait limit). Later PE ops are same-engine ordered."""
                ps = pst_p.tile([128, ST_TOK], F32, tag="ps")
                a = ap_a[0:1, 0:1].bitcast(BF16)
                b = ap_b[0:1, 0:1].bitcast(BF16)
                nc.tensor.matmul(ps[0:a.free_size(), 0:b.free_size()],
                                 a, b, start=True, stop=True,
                                 skip_group_check=True)
            # --- persistent weights ---
            At_sb = consts.tile([128, NSTACKS * NJ * 128], a_dt)  # part=c, free=(k,j,m)
            nc.sync.dma_start(
                out=At_sb,
                in_=bass.AP(tensor=At_d, offset=0,
                            ap=[[128, 128],                    # c (partition)
                                [128 * 128, NSTACKS * NJ],     # (k,j) merged
                                [1, 128]]),                    # m
            )
            Bw_sb = consts.tile([128, NSTACKS * NG * 128], BF16)  # part=q, free=(k,g,of)
            nc.sync.dma_start(
                out=Bw_sb,
                in_=bass.AP(tensor=Bw_d, offset=0,
                            ap=[[128, 128],
                                [128 * 128, NSTACKS * NG],
                                [1, 128]]),
            )
            b4_sb = consts.tile([1, NSTACKS * NG * 128], BF16)
            nc.sync.dma_start(out=b4_sb, in_=b4_d[:].flatten().unsqueeze(0))
            id_sb = consts.tile([128, 128], F32)
            nc.sync.dma_start(out=id_sb, in_=id_d[:])
            ones_sb = consts.tile([1, 128], BF16)
            nc.vector.memset(ones_sb, 1.0)

            def load_x(st):
                x_tm = xtm_p.tile([128, NCH * N_], F32, tag="x_tm")
                dma = nc.gpsimd.dma_start(
                    out=x_tm,
                    in_=bass.AP(tensor=x_d, offset=st * ST_TOK * N_,
                                ap=[[N_, 128],          # p (partition)
                                    [128 * N_, NCH],    # c
                                    [1, N_]]),          # f
                )
                return x_tm, dma

            last = {"pe": None, "xdma": None, "odma": None,
                    "ybf": None, "gelu": None, "bmm": None}

            x_tm, last["xdma"] = load_x(0)
            for st in range(NSUP):
                # --- transpose to feature-major: x_fm [128 f_in_block, (j, t)] ---
                x_fm = xfm_p.tile([128, NJ * ST_TOK], a_dt)
                for j in range(NJ):
                    ps = pst_p.tile([128, ST_TOK], F32)
                    for c in range(NCH):
                        tr = nc.tensor.transpose(
                            ps[:, 128 * c:128 * (c + 1)],
                            x_tm[:, c * N_ + 128 * j: c * N_ + 128 * (j + 1)],
                            id_sb,
                        )
                    last["pe"] = tr
                    nc.vector.tensor_copy(x_fm[:, ST_TOK * j:ST_TOK * (j + 1)], ps)

                # prefetch next supertile (x_tm readers are done after transposes)
                if st + 1 < NSUP:
                    x_tm, last["xdma"] = load_x(st + 1)

                out_sb = outb_p.tile([128, NSTACKS * NCH * N_], out_dt, tag="outsb")
                for k in range(NSTACKS):
                    # --- stage A ---
                    y_bf = ybf_p.tile([128, NJ * ST_TOK], BF16)
                    for j in range(NJ):
                        yps = psy_p.tile([128, ST_TOK], F32)
                        nc.tensor.matmul(
                            yps,
                            At_sb[:, (k * NJ + j) * 128:(k * NJ + j + 1) * 128],
                            x_fm[:, ST_TOK * j:ST_TOK * (j + 1)],
                            start=True, stop=True,
                        )
                        last["ybf"] = nc.vector.tensor_copy(
                            y_bf[:, ST_TOK * j:ST_TOK * (j + 1)], yps)

                    # --- shuffle: block-grouping -> residue-grouping (bf16) ---
                    # one DMA per residue-group g: y_sh[16j+a, 512g+t] = y_bf[16g+a, 512j+t]
                    y_sh = ysh_p.tile([128, NG * ST_TOK], BF16)
                    for g in range(NG):
                        for j in range(NJ):
                            eng = nc.scalar if (g + j) % 2 else nc.sync
                            eng.dma_start(
                                out=y_sh[16 * j:16 * (j + 1),
                                         ST_TOK * g:ST_TOK * (g + 1)],
                                in_=y_bf[16 * g:16 * (g + 1),
                                         ST_TOK * j:ST_TOK * (j + 1)])

                    # --- stage B (flipped) + bias preload + gelu ---
                    for g in range(NG):
                        pso = pso_p.tile([128, ST_TOK], F32)
                        bias_rhs = b4_sb[:, (k * NG + g) * 128:(k * NG + g + 1) * 128]
                        for c in range(NCH):
                            reg = pso[:, 128 * c:128 * (c + 1)]
                            nc.tensor.matmul(
                                reg, ones_sb, bias_rhs,
                                start=True, stop=False, skip_group_check=True,
                            )
                            last["bmm"] = nc.tensor.matmul(
                                reg,
                                y_sh[:, ST_TOK * g + 128 * c: ST_TOK * g + 128 * (c + 1)],
                                Bw_sb[:, (k * NG + g) * 128:(k * NG + g + 1) * 128],
                                start=False, stop=True,
                                skip_group_check=True,
                            )

                        # gelu: PSUM [128 t, (c, of=(j',a))] -> out_sb strided
                        dst = bass.AP(
                            tensor=out_sb.tensor,
                            offset=out_sb.offset + k * (NCH * N_) + 16 * g,
                            ap=[[NSTACKS * NCH * N_, 128],  # partition (t)
                                [N_, NCH],                  # c
                                [128, NJ],                  # j'
                                [1, 16]])                   # a
                        act_fn = (mybir.ActivationFunctionType.Gelu
                                  if GELU == "gelu"
                                  else mybir.ActivationFunctionType.Copy)
                        last["gelu"] = nc.scalar.activation(dst, pso, act_fn)

                # --- store supertile: cast bf16 -> f32 on SWDGE, one DMA per c ---
                # out_sb free = (k, c, f); HBM token = 512 st + 128 c + p
                for c in range(NCH):
                    src = bass.AP(tensor=out_sb.tensor,
                                  offset=out_sb.offset + c * N_,
                                  ap=[[NSTACKS * NCH * N_, 128],  # p
                                      [NCH * N_, NSTACKS],        # k
                                      [1, N_]])                   # f
                    dst = bass.AP(tensor=out_d,
                                  offset=(st * ST_TOK + c * 128) * (NSTACKS * N_),
                                  ap=[[NSTACKS * N_, 128],        # p
                                      [N_, NSTACKS],              # k
                                      [1, N_]])                   # f
                    if out_dt is F32:
                        last["odma"] = nc.sync.dma_start(out=dst, in_=src)
                    else:
                        last["odma"] = nc.gpsimd.dma_start(out=dst, in_=src)
    nc.finalize()
    return nc


_NC_CACHE = None


def kernel(hidden_states, twiddle, bias):
    global _NC_CACHE
    x = np.ascontiguousarray(np.asarray(hidden_states, np.float32)).reshape(TOK, N_)
    At, Bw, bias4 = _factor_weights(twiddle, bias)
    Bw16 = Bw.astype(np.dtype("bfloat16")) if False else Bw  # cast below via ml_dtypes
    import ml_dtypes
    Bw16 = Bw.astype(ml_dtypes.bfloat16)
    ident = np.eye(128, dtype=np.float32)

    if _NC_CACHE is None:
        _NC_CACHE = build_kernel()
    nc = _NC_CACHE

    in_maps = []
    for i in range(NCORES):
        in_maps.append({
            "x": np.ascontiguousarray(x[i * TPC:(i + 1) * TPC]),
            "At": At, "Bw": Bw16, "bias4": bias4.astype(ml_dtypes.bfloat16), "ident": ident,
        })
    res = bass_utils.run_bass_kernel_spmd(nc, in_maps, core_ids=list(range(NCORES)))
    global LAST_RESULT
    LAST_RESULT = res
    out = np.concatenate([res.results[i]["out"] for i in range(NCORES)], axis=0)
    return out.reshape(B_, S_, NSTACKS * N_)


LAST_RESULT = None


if __name__ == "__main__":
    rng = np.random.default_rng(0)
    h = rng.standard_normal((B_, S_, N_), dtype=np.float32)
    tw = (rng.standard_normal((NSTACKS, LOG_N, N_ // 2, 2, 2)) * 2 ** -0.5).astype(np.float32)
    b = rng.standard_normal(NSTACKS * N_).astype(np.float32)
    out = kernel(h, tw, b)
    print("out", out.shape, out.dtype, np.abs(out).max())
